# revision 11
# baseline (speedup 1.0000x reference)
"""DIN (sparse-attention) Trainium2 kernel.

Sharding: data-parallel over batch across 8 cores (1024 rows each).
Embedding tables are replicated (seq tables concatenated + cast to bf16 on
host, a value-level transform; all index-dependent gathering happens on
device via indirect DMA). Attention MLP runs in bf16 (weights folded:
[q,s,q-s,q*s]@W1 == q@(W1a+W1c) + s@(W1b-W1c) + (q*s)@W1d); BatchNorm is
folded into the FFN's first layer; the FFN runs in fp32r (TF32-rate
matmuls). Final attention bias is dropped (softmax shift invariance) and
softmax runs without max subtraction (logits are tiny by construction).
"""
import sys
sys.path.insert(0, '/opt/trn_rl_repo')

import numpy as np
import ml_dtypes

import bass_rust
import concourse.bass as bass
import concourse.bacc as bacc
import concourse.mybir as mybir
import concourse.tile as tile_mod
from concourse.tile import TileContext
from concourse.bass_utils import run_bass_kernel_spmd
from concourse.masks import make_identity
from concourse.alu_op_type import AluOpType
from concourse.vector_clock import ScopedClock

BF16NP = ml_dtypes.bfloat16
F32 = mybir.dt.float32
F32R = mybir.dt.float32r
BF16 = mybir.dt.bfloat16
I32 = mybir.dt.int32

B, L, NB = 8192, 40, 2
V, E = 100000, 64
NS, ND = 3, 8
NCORES = 8
BC = B // NCORES            # 1024 batch rows per core
NBT = BC // 128             # 8 batch tiles of 128
T = BC * L                  # 40960 tokens per core

# ---------------------------------------------------------------------------
# Patches for this container's walrus build (max ONE sync-wait per
# instruction): split multi-wait instructions into nop chains, and make the
# TileContext exit drain use the same discipline.
_split_ctr = [0]


def _split_multi_waits(nc):
    for fn in nc.m.functions:
        for blk in fn.blocks:
            insts = blk.instructions
            i = 0
            while i < len(insts):
                inst = insts[i]
                si = inst.sync_info
                if si is None:
                    i += 1
                    continue
                waits = list(si.on_wait or [])
                ups = list(si.on_update or [])
                if len(waits) <= 1 and len(ups) <= 1:
                    i += 1
                    continue
                inst.sync_info = bass_rust.SyncInfo(
                    on_wait=waits[-1:], on_update=ups[:1])
                pre = []
                for w in waits[:-1]:
                    _split_ctr[0] += 1
                    n = mybir.InstNoOp(name=f"waitsplit-{_split_ctr[0]}",
                                       ins=[], outs=[])
                    n.engine = inst.engine
                    n.sync_info = bass_rust.SyncInfo(on_wait=[w], on_update=[])
                    pre.append(n)
                post = []
                for u in ups[1:]:
                    _split_ctr[0] += 1
                    n = mybir.InstNoOp(name=f"upsplit-{_split_ctr[0]}",
                                       ins=[], outs=[])
                    n.engine = inst.engine
                    n.sync_info = bass_rust.SyncInfo(on_wait=[], on_update=[u])
                    post.append(n)
                insts[i:i] = pre
                insts[i + len(pre) + 1:i + len(pre) + 1] = post
                i += len(pre) + 1 + len(post)


def _patched_drain_and_barrier(self, tick_clock, wait_clock):
    nc = self.nc
    probe = nc.sync.nop(nofuse=True, hint="drain_wait_split")
    wait_clock.add_sem_waits(
        probe.ins, ScopedClock({None: tick_clock.global_clock}))
    si = probe.ins.sync_info
    waits = list(si.on_wait) if si is not None and si.on_wait else []
    if len(waits) > 1:
        si.on_wait = [waits[0]]
        for w in waits[1:]:
            n = nc.sync.nop(nofuse=True, hint="drain_wait_split")
            n.ins.sync_info = bass_rust.SyncInfo(on_wait=[w], on_update=[])
    nc.sync.drain()
    nc.all_engine_barrier()
    assert self.sems is not None
    popped = nc._tile_sem_poison_stack.pop()
    assert popped is self._sem_poison
    nc.clear_and_free_semaphores(list(self.sems.allocated().values()))
    nc.all_engine_barrier()


tile_mod.TileContext._drain_and_barrier = _patched_drain_and_barrier


# ---------------------------------------------------------------------------
def _build_nc():
    nc = bacc.Bacc()
    dt = nc.dram_tensor
    emb_cat = dt("emb_cat", [2 * V, E], BF16, kind="ExternalInput")
    sp_tabs = [dt(f"sp{i}", [V, E], F32, kind="ExternalInput")
               for i in range(NS)]
    idx_seq = dt("idx_seq", [128, 2 * L * NBT], I32, kind="ExternalInput")
    idx_item = dt("idx_item", [128, 2 * NBT], I32, kind="ExternalInput")
    idx_sp = dt("idx_sp", [128, NS * NBT], I32, kind="ExternalInput")
    idx0lb = dt("idx0lb", [L, BC], I32, kind="ExternalInput")
    dense_t = dt("dense_t", [ND, BC], F32R, kind="ExternalInput")
    aW1 = dt("aW1", [128, 3, 80], BF16, kind="ExternalInput")
    aB1 = dt("aB1", [80, 1], F32, kind="ExternalInput")
    aW2 = dt("aW2", [80, 40], BF16, kind="ExternalInput")
    aB2 = dt("aB2", [40, 1], F32, kind="ExternalInput")
    aWf = dt("aWf", [40, 1], BF16, kind="ExternalInput")
    fW1 = dt("fW1", [456, 80], F32R, kind="ExternalInput")
    fB1 = dt("fB1", [80, 1], F32, kind="ExternalInput")
    fA1 = dt("fA1", [80, 1], F32, kind="ExternalInput")
    fW2 = dt("fW2", [80, 40], F32R, kind="ExternalInput")
    fB2 = dt("fB2", [40, 1], F32, kind="ExternalInput")
    fA2 = dt("fA2", [40, 1], F32, kind="ExternalInput")
    fWo = dt("fWo", [40, 1], F32R, kind="ExternalInput")
    fBo = dt("fBo", [1, 1], F32, kind="ExternalInput")
    selS = dt("selS", [L, 128 * L], BF16, kind="ExternalInput")
    ones40 = dt("ones40", [L, 1], BF16, kind="ExternalInput")
    onesK1 = dt("onesK1", [1, 128], F32, kind="ExternalInput")
    y = dt("y", [BC, 1], F32, kind="ExternalOutput")

    with TileContext(nc) as tc:
        with tc.tile_pool(name="sb", bufs=1) as pool:
            # --- static loads -------------------------------------------
            t_idx_seq = pool.tile([128, 2 * L * NBT], I32, tag="t_idx_seq")
            t_idx_item = pool.tile([128, 2 * NBT], I32, tag="t_idx_item")
            t_idx_sp = pool.tile([128, NS * NBT], I32, tag="t_idx_sp")
            t_idx0 = pool.tile([L, BC], I32, tag="t_idx0")
            nc.sync.dma_start(t_idx_seq[:], idx_seq[:])
            nc.sync.dma_start(t_idx_item[:], idx_item[:])
            nc.sync.dma_start(t_idx_sp[:], idx_sp[:])
            nc.sync.dma_start(t_idx0[:], idx0lb[:])

            t_aW1 = pool.tile([128, 3, 80], BF16, tag="t_aW1")
            t_aB1 = pool.tile([80, 1], F32, tag="t_aB1")
            t_aW2 = pool.tile([80, 40], BF16, tag="t_aW2")
            t_aB2 = pool.tile([40, 1], F32, tag="t_aB2")
            t_aWf = pool.tile([40, 1], BF16, tag="t_aWf")
            for t, s in [(t_aW1, aW1), (t_aB1, aB1), (t_aW2, aW2),
                         (t_aB2, aB2), (t_aWf, aWf)]:
                nc.sync.dma_start(t[:], s[:])
            t_fW1 = [pool.tile([128, 80], F32R, tag=f"t_fW1_{k}", name=f"t_fW1_{k}")
                     for k in range(3)] + [pool.tile([72, 80], F32R, tag="t_fW1_3", name="t_fW1_3")]
            for k in range(3):
                nc.sync.dma_start(t_fW1[k][:], fW1[128 * k:128 * (k + 1), :])
            nc.sync.dma_start(t_fW1[3][:], fW1[384:456, :])
            t_fB1 = pool.tile([80, 1], F32, tag="t_fB1")
            t_fA1 = pool.tile([80, 1], F32, tag="t_fA1")
            t_fW2 = pool.tile([80, 40], F32R, tag="t_fW2")
            t_fB2 = pool.tile([40, 1], F32, tag="t_fB2")
            t_fA2 = pool.tile([40, 1], F32, tag="t_fA2")
            t_fWo = pool.tile([40, 1], F32R, tag="t_fWo")
            t_fBo = pool.tile([1, 1], F32, tag="t_fBo")
            for t, s in [(t_fB1, fB1), (t_fA1, fA1), (t_fW2, fW2),
                         (t_fB2, fB2), (t_fA2, fA2), (t_fWo, fWo),
                         (t_fBo, fBo)]:
                nc.sync.dma_start(t[:], s[:])

            idb = pool.tile([128, 128], BF16, tag="idb")
            idf = pool.tile([128, 128], F32, tag="idf")
            make_identity(nc, idb[:])
            make_identity(nc, idf[:])

            # xT3 holds sparse2^T plus dense^T rows; dense lands directly.
            xT = [pool.tile([128, BC], F32R, tag=f"xT{k}", name=f"xT{k}") for k in range(3)]
            xT.append(pool.tile([72, BC], F32R, tag="xT3", name="xT3"))
            nc.sync.dma_start(xT[3][64:72, :], dense_t[:])

            # persistent gathered data
            seq = pool.tile([128, L * NBT, 2 * E], BF16, tag="seq")
            item = pool.tile([128, 2 * NBT, E], BF16, tag="item")
            spr = pool.tile([128, NS * NBT, E], F32, tag="spr")

            maskF = pool.tile([L, BC], BF16, tag="maskF")
            nc.vector.tensor_scalar(out=maskF[:], in0=t_idx0[:], scalar1=0,
                                    scalar2=None, op0=AluOpType.not_equal)

            t_selS = pool.tile([L, 128 * L], BF16, tag="t_selS")
            t_ones40 = pool.tile([L, 1], BF16, tag="t_ones40")
            t_onesK1 = pool.tile([1, 128], F32, tag="t_onesK1")
            nc.sync.dma_start(t_selS[:], selS[:])
            nc.sync.dma_start(t_ones40[:], ones40[:])
            nc.sync.dma_start(t_onesK1[:], onesK1[:])
            emLB = pool.tile([L, BC], BF16, tag="emLB")
            nc.vector.memset(emLB[:], 0.0)
            uiT_un = pool.tile([128, BC], F32, tag="uiT_un")

            # --- gathers: item + sparse (small) --------------------------
            for c in range(2 * NBT):
                nc.gpsimd.indirect_dma_start(
                    out=item[:, c, :], out_offset=None, in_=emb_cat[:],
                    in_offset=bass.IndirectOffsetOnAxis(
                        ap=t_idx_item[:, c:c + 1], axis=0))

            for si in range(NS):
                for bt in range(NBT):
                    c = si * NBT + bt
                    nc.gpsimd.indirect_dma_start(
                        out=spr[:, c, :], out_offset=None, in_=sp_tabs[si][:],
                        in_offset=bass.IndirectOffsetOnAxis(
                            ap=t_idx_sp[:, c:c + 1], axis=0))

            with tc.tile_pool(name="psA", bufs=1, space="PSUM") as psA:
                # qT: [128 feat, BC] bf16 from item tiles
                qT = pool.tile([128, BC], BF16, tag="qT")
                for tb in range(2):
                    for bt in range(NBT):
                        pt = psA.tile([128, 128], BF16, tag="ptrans", bufs=2)
                        nc.tensor.transpose(
                            pt[0:64, :], item[:, tb * NBT + bt, :], idb[:])
                        nc.vector.tensor_copy(
                            qT[64 * tb:64 * (tb + 1),
                               bt * 128:(bt + 1) * 128], pt[0:64, :])

                # --- main attention loop over l --------------------------
                for l in range(L):
                    for tb in range(2):
                        for bt in range(NBT):
                            j = l * NBT + bt
                            nc.gpsimd.indirect_dma_start(
                                out=seq[:, j, tb * E:(tb + 1) * E],
                                out_offset=None, in_=emb_cat[:],
                                in_offset=bass.IndirectOffsetOnAxis(
                                    ap=t_idx_seq[:, tb * L * NBT + j:
                                                 tb * L * NBT + j + 1],
                                    axis=0))
                    sT = pool.tile([128, BC], BF16, tag="sT", bufs=3)
                    for bt in range(NBT):
                        pt = psA.tile([128, 128], BF16, tag="ptrans", bufs=2)
                        nc.tensor.transpose(pt[:], seq[:, l * NBT + bt, :],
                                            idb[:])
                        nc.vector.tensor_copy(
                            sT[:, bt * 128:(bt + 1) * 128], pt[:])
                    qs = pool.tile([128, BC], BF16, tag="qs", bufs=2)
                    h1 = pool.tile([80, BC], BF16, tag="h1", bufs=2)
                    h2 = pool.tile([40, BC], BF16, tag="h2", bufs=2)
                    pfl = psA.tile([33, 512], F32, tag="pfl", bufs=1)
                    ftmp = pool.tile([33, 512], BF16, tag="ftmp", bufs=2)
                    for ns in range(2):
                        sl = slice(512 * ns, 512 * (ns + 1))
                        nc.vector.tensor_tensor(
                            out=qs[:, sl], in0=sT[:, sl], in1=qT[:, sl],
                            op=AluOpType.mult)
                        p1 = psA.tile([80, 512], F32, tag="pmm1", bufs=2)
                        nc.tensor.matmul(p1[:], t_aW1[:, 0, :], qT[:, sl],
                                         start=True, stop=False)
                        nc.tensor.matmul(p1[:], t_aW1[:, 1, :], sT[:, sl],
                                         start=False, stop=False)
                        nc.tensor.matmul(p1[:], t_aW1[:, 2, :], qs[:, sl],
                                         start=False, stop=True)
                        nc.scalar.activation(
                            h1[:, sl], p1[:],
                            mybir.ActivationFunctionType.Sigmoid,
                            bias=t_aB1[:])
                        p2 = psA.tile([40, 512], F32, tag="pmm2", bufs=1)
                        nc.tensor.matmul(p2[:], t_aW2[:], h1[:, sl],
                                         start=True, stop=True)
                        nc.scalar.activation(
                            h2[:, sl], p2[:],
                            mybir.ActivationFunctionType.Sigmoid,
                            bias=t_aB2[:])
                        nc.tensor.matmul(pfl[32 * ns:32 * ns + 1, :],
                                         t_aWf[:], h2[:, sl],
                                         start=True, stop=True,
                                         tile_position=(0, 32 * ns))
                    m0 = pool.tile([33, 512], BF16, tag="m0", bufs=2,
                                   name="m0")
                    nc.sync.dma_start(m0[0:1, :], maskF[l:l + 1, 0:512])
                    nc.sync.dma_start(m0[32:33, :],
                                      maskF[l:l + 1, 512:1024])
                    nc.scalar.activation(ftmp[:], pfl[:],
                                         mybir.ActivationFunctionType.Exp)
                    nc.vector.tensor_tensor(out=ftmp[0:1, :],
                                            in0=ftmp[0:1, :],
                                            in1=m0[0:1, :],
                                            op=AluOpType.mult)
                    nc.vector.tensor_tensor(out=ftmp[32:33, :],
                                            in0=ftmp[32:33, :],
                                            in1=m0[32:33, :],
                                            op=AluOpType.mult)
                    nc.sync.dma_start(emLB[l:l + 1, 0:512], ftmp[0:1, :])
                    nc.sync.dma_start(emLB[l:l + 1, 512:1024],
                                      ftmp[32:33, :])
                    for ns in range(2):
                        sl = slice(512 * ns, 512 * (ns + 1))
                        pe = psA.tile([128, 512], F32, tag="pemB", bufs=2,
                                      name="pemB")
                        nc.tensor.matmul(pe[:],
                                         t_selS[:, 128 * l:128 * (l + 1)],
                                         emLB[:, sl], start=True, stop=True)
                        emBsb = pool.tile([128, 512], BF16, tag="emBsb",
                                          bufs=2, name="emBsb")
                        nc.vector.tensor_copy(emBsb[:], pe[:])
                        uit = pool.tile([128, 512], F32, tag="uitmp",
                                        bufs=2, name="uitmp")
                        nc.vector.tensor_tensor(out=uit[:], in0=sT[:, sl],
                                                in1=emBsb[:],
                                                op=AluOpType.mult)
                        if l == 0:
                            nc.vector.tensor_copy(uiT_un[:, sl], uit[:])
                        else:
                            nc.vector.tensor_tensor(
                                out=uiT_un[:, sl], in0=uiT_un[:, sl],
                                in1=uit[:], op=AluOpType.add)

            # --- softmax denominator + normalize (feature-major) ---------
            with tc.tile_pool(name="psB", bufs=1, space="PSUM") as psB:
                rden0 = pool.tile([1, BC], F32, tag="rden0")
                for ns in range(2):
                    sl = slice(512 * ns, 512 * (ns + 1))
                    pden = psB.tile([1, 512], F32, tag="p1x512", bufs=2,
                                    name="pden")
                    nc.tensor.matmul(pden[:], t_ones40[:], emLB[:, sl],
                                     start=True, stop=True)
                    nc.vector.reciprocal(rden0[0:1, sl], pden[:])
                for ns in range(2):
                    sl = slice(512 * ns, 512 * (ns + 1))
                    prb = psB.tile([128, 512], F32, tag="prdenB", bufs=1,
                                   name="prb")
                    nc.tensor.matmul(prb[:], t_onesK1[:], rden0[0:1, sl],
                                     start=True, stop=True)
                    nc.vector.tensor_tensor(out=xT[0][:, sl],
                                            in0=uiT_un[:, sl], in1=prb[:],
                                            op=AluOpType.mult)
                nc.vector.tensor_copy(xT[1][:], qT[:])
                for si in range(NS):
                    for bt in range(NBT):
                        pu = psB.tile([128, 128], F32, tag="puiT", bufs=1)
                        nc.tensor.transpose(
                            pu[0:64, :], spr[:, si * NBT + bt, :], idf[:])
                        dst = (xT[2][64 * si:64 * (si + 1),
                                     bt * 128:(bt + 1) * 128] if si < 2
                               else xT[3][0:64, bt * 128:(bt + 1) * 128])
                        nc.vector.tensor_copy(dst, pu[0:64, :])

                # --- FFN ------------------------------------------------
                y_sb = pool.tile([1, BC], F32, tag="y_sb")
                fh1 = pool.tile([80, BC], F32R, tag="fh1")
                fh2 = pool.tile([40, BC], F32R, tag="fh2")
                tmp_b = pool.tile([80, 512], F32, tag="tmp_b", bufs=2)
                tmp_n = pool.tile([80, 512], F32, tag="tmp_n", bufs=2)
                tmp_p = pool.tile([80, 512], F32, tag="tmp_p", bufs=2)
                for ns in range(2):
                    sl = slice(512 * ns, 512 * (ns + 1))
                    pf1 = psB.tile([80, 512], F32, tag="pffn1", bufs=2)
                    nc.tensor.matmul(pf1[:], t_fW1[0][:], xT[0][:, sl],
                                     start=True, stop=False)
                    nc.tensor.matmul(pf1[:], t_fW1[1][:], xT[1][:, sl],
                                     start=False, stop=False)
                    nc.tensor.matmul(pf1[:], t_fW1[2][:], xT[2][:, sl],
                                     start=False, stop=False)
                    nc.tensor.matmul(pf1[:], t_fW1[3][:], xT[3][:, sl],
                                     start=False, stop=True)
                    bq = pool.tile([80, 512], F32, tag="bq", bufs=2)
                    nc.vector.tensor_scalar(
                        out=bq[:], in0=pf1[:], scalar1=t_fB1[:],
                        scalar2=None, op0=AluOpType.add)
                    nc.vector.tensor_scalar(out=tmp_n[:80], in0=bq[:],
                                            scalar1=0.0, scalar2=None,
                                            op0=AluOpType.min)
                    nc.vector.tensor_scalar(out=tmp_p[:80], in0=bq[:],
                                            scalar1=0.0, scalar2=None,
                                            op0=AluOpType.max)
                    nc.vector.scalar_tensor_tensor(
                        out=fh1[:, sl], in0=tmp_n[:80], scalar=t_fA1[:],
                        in1=tmp_p[:80], op0=AluOpType.mult,
                        op1=AluOpType.add)
                    pf2 = psB.tile([40, 512], F32, tag="pffn2", bufs=1)
                    nc.tensor.matmul(pf2[:], t_fW2[:], fh1[:, sl],
                                     start=True, stop=True)
                    bq2 = pool.tile([40, 512], F32, tag="bq2", bufs=2)
                    nc.vector.tensor_scalar(
                        out=bq2[:], in0=pf2[:], scalar1=t_fB2[:],
                        scalar2=None, op0=AluOpType.add)
                    nc.vector.tensor_scalar(out=tmp_n[:40], in0=bq2[:],
                                            scalar1=0.0, scalar2=None,
                                            op0=AluOpType.min)
                    nc.vector.tensor_scalar(out=tmp_p[:40], in0=bq2[:],
                                            scalar1=0.0, scalar2=None,
                                            op0=AluOpType.max)
                    nc.vector.scalar_tensor_tensor(
                        out=fh2[:, sl], in0=tmp_n[:40], scalar=t_fA2[:],
                        in1=tmp_p[:40], op0=AluOpType.mult,
                        op1=AluOpType.add)
                    pf3 = psB.tile([1, 512], F32, tag="pffn3", bufs=1)
                    nc.tensor.matmul(pf3[:], t_fWo[:], fh2[:, sl],
                                     start=True, stop=True)
                    nc.scalar.activation(
                        y_sb[0:1, sl], pf3[:],
                        mybir.ActivationFunctionType.Sigmoid,
                        bias=t_fBo[:])
            nc.sync.dma_start(y[:].rearrange("a b -> b a"), y_sb[:])

    nc.compile()
    _split_multi_waits(nc)
    bass.Bass.finalize(nc)
    return nc


_NC_CACHE = []


def _get_nc():
    if not _NC_CACHE:
        _NC_CACHE.append(_build_nc())
    return _NC_CACHE[0]


_SELS = np.zeros((L, 128 * L), np.float32)
for _l in range(L):
    _SELS[_l, 128 * _l:128 * (_l + 1)] = 1.0


def _prep_shared(inputs):
    emb_cat = np.ascontiguousarray(
        np.concatenate([np.asarray(inputs["emb_seq"][0]),
                        np.asarray(inputs["emb_seq"][1])], axis=0)
    ).astype(BF16NP)
    sp = [np.ascontiguousarray(np.asarray(inputs["emb_sparse"][i]),
                               dtype=np.float32) for i in range(NS)]

    W1 = np.asarray(inputs["att_W1"], np.float32)          # [512, 80]
    W1q = W1[0:128] + W1[256:384]
    W1s = W1[128:256] - W1[256:384]
    W1qs = W1[384:512]
    aW1 = np.stack([W1q, W1s, W1qs], axis=1).astype(BF16NP)  # [128, 3, 80]
    aB1 = np.asarray(inputs["att_b1"], np.float32).reshape(80, 1)
    aW2 = np.asarray(inputs["att_W2"], np.float32).astype(BF16NP)
    aB2 = np.asarray(inputs["att_b2"], np.float32).reshape(40, 1)
    aWf = np.asarray(inputs["att_Wf"], np.float32).astype(BF16NP)

    gamma = np.asarray(inputs["bn_gamma"], np.float32)
    beta = np.asarray(inputs["bn_beta"], np.float32)
    mean = np.asarray(inputs["bn_mean"], np.float32)
    var = np.asarray(inputs["bn_var"], np.float32)
    scale = gamma / np.sqrt(var + 1e-3)
    shift = beta - mean * scale
    fW1o = np.asarray(inputs["ffn_W1"], np.float32)        # [456, 80]
    W1f = fW1o * scale[:, None]
    b1f = shift @ fW1o + np.asarray(inputs["ffn_b1"], np.float32)
    # reorder rows [ui(0:128), item(128:256), dense(256:264), sparse(264:456)]
    # -> [ui, item, sparse, dense]
    perm = np.concatenate([np.arange(0, 256), np.arange(264, 456),
                           np.arange(256, 264)])
    fW1 = np.ascontiguousarray(W1f[perm])
    return dict(
        emb_cat=emb_cat, sp=sp, aW1=aW1, aB1=aB1, aW2=aW2, aB2=aB2, aWf=aWf,
        fW1=fW1, fB1=b1f.reshape(80, 1),
        fA1=np.asarray(inputs["ffn_a1"], np.float32).reshape(80, 1),
        fW2=np.asarray(inputs["ffn_W2"], np.float32),
        fB2=np.asarray(inputs["ffn_b2"], np.float32).reshape(40, 1),
        fA2=np.asarray(inputs["ffn_a2"], np.float32).reshape(40, 1),
        fWo=np.asarray(inputs["out_W"], np.float32),
        fBo=np.asarray(inputs["out_b"], np.float32).reshape(1, 1),
        selS=_SELS.astype(BF16NP), ones40=np.ones((L, 1), np.float32).astype(BF16NP),
        onesK1=np.ones((1, 128), np.float32),
    )


def _prep_core(inputs, ci):
    s = slice(ci * BC, (ci + 1) * BC)
    seq = np.asarray(inputs["seq_inputs"])[s]      # [1024, 40, 2]
    itm = np.asarray(inputs["item_inputs"])[s]     # [1024, 2]
    spi = np.asarray(inputs["sparse_inputs"])[s]   # [1024, 3]
    dns = np.asarray(inputs["dense_inputs"])[s]    # [1024, 8]

    idx_seq = np.empty((128, 2 * L * NBT), np.int32)
    for tb in range(2):
        # [1024, 40] -> cols (l*8 + bt), partition = b % 128
        v = seq[:, :, tb].reshape(NBT, 128, L) + tb * V
        idx_seq[:, tb * L * NBT:(tb + 1) * L * NBT] = (
            v.transpose(1, 2, 0).reshape(128, L * NBT))
    idx_item = np.empty((128, 2 * NBT), np.int32)
    for tb in range(2):
        idx_item[:, tb * NBT:(tb + 1) * NBT] = (
            itm[:, tb].reshape(NBT, 128).T + tb * V)
    idx_sp = np.empty((128, NS * NBT), np.int32)
    for si in range(NS):
        idx_sp[:, si * NBT:(si + 1) * NBT] = spi[:, si].reshape(NBT, 128).T
    idx0lb = np.ascontiguousarray(seq[:, :, 0].T.astype(np.int32))  # [40,1024]
    dense_t = np.ascontiguousarray(dns.T)                  # [8, 1024]
    return dict(idx_seq=idx_seq, idx_item=idx_item, idx_sp=idx_sp,
                idx0lb=idx0lb, dense_t=dense_t)


def kernel(**inputs):
    nc = _get_nc()
    sh = _prep_shared(inputs)
    in_maps = []
    for ci in range(NCORES):
        pc = _prep_core(inputs, ci)
        m = {
            "emb_cat": np.asarray(sh["emb_cat"]),
            "sp0": sh["sp"][0], "sp1": sh["sp"][1], "sp2": sh["sp"][2],
            "idx_seq": pc["idx_seq"], "idx_item": pc["idx_item"],
            "idx_sp": pc["idx_sp"], "idx0lb": pc["idx0lb"],
            "dense_t": pc["dense_t"],
            "aW1": np.asarray(sh["aW1"]), "aB1": sh["aB1"],
            "aW2": np.asarray(sh["aW2"]), "aB2": sh["aB2"],
            "aWf": np.asarray(sh["aWf"]),
            "fW1": sh["fW1"], "fB1": sh["fB1"], "fA1": sh["fA1"],
            "fW2": sh["fW2"], "fB2": sh["fB2"], "fA2": sh["fA2"],
            "fWo": sh["fWo"], "fBo": sh["fBo"],
            "selS": np.asarray(sh["selS"]), "ones40": np.asarray(sh["ones40"]),
            "onesK1": sh["onesK1"],
        }
        in_maps.append(m)
    res = run_bass_kernel_spmd(nc, in_maps, core_ids=list(range(NCORES)))
    out = np.concatenate([res.results[ci]["y"] for ci in range(NCORES)],
                         axis=0)
    return out


# revision 12
# speedup vs baseline: 1.0369x; 1.0369x over previous
"""DIN (sparse-attention) Trainium2 kernel.

Sharding: data-parallel over batch across 8 cores (1024 rows each).
Embedding tables are replicated (seq tables concatenated + cast to bf16 on
host, a value-level transform; all index-dependent gathering happens on
device via indirect DMA). Attention MLP runs in bf16 (weights folded:
[q,s,q-s,q*s]@W1 == q@(W1a+W1c) + s@(W1b-W1c) + (q*s)@W1d); BatchNorm is
folded into the FFN's first layer; the FFN runs in fp32r (TF32-rate
matmuls). Final attention bias is dropped (softmax shift invariance) and
softmax runs without max subtraction (logits are tiny by construction).
"""
import sys
sys.path.insert(0, '/opt/trn_rl_repo')

import numpy as np
import ml_dtypes

import bass_rust
import concourse.bass as bass
import concourse.bacc as bacc
import concourse.mybir as mybir
import concourse.tile as tile_mod
from concourse.tile import TileContext
from concourse.bass_utils import run_bass_kernel_spmd
from concourse.masks import make_identity
from concourse.alu_op_type import AluOpType
from concourse.vector_clock import ScopedClock

BF16NP = ml_dtypes.bfloat16
F32 = mybir.dt.float32
F32R = mybir.dt.float32r
BF16 = mybir.dt.bfloat16
I32 = mybir.dt.int32

B, L, NB = 8192, 40, 2
V, E = 100000, 64
NS, ND = 3, 8
NCORES = 8
BC = B // NCORES            # 1024 batch rows per core
NBT = BC // 128             # 8 batch tiles of 128
T = BC * L                  # 40960 tokens per core

# ---------------------------------------------------------------------------
# Patches for this container's walrus build (max ONE sync-wait per
# instruction): split multi-wait instructions into nop chains, and make the
# TileContext exit drain use the same discipline.
_split_ctr = [0]


def _split_multi_waits(nc):
    for fn in nc.m.functions:
        for blk in fn.blocks:
            insts = blk.instructions
            i = 0
            while i < len(insts):
                inst = insts[i]
                si = inst.sync_info
                if si is None:
                    i += 1
                    continue
                waits = list(si.on_wait or [])
                ups = list(si.on_update or [])
                if len(waits) <= 1 and len(ups) <= 1:
                    i += 1
                    continue
                inst.sync_info = bass_rust.SyncInfo(
                    on_wait=waits[-1:], on_update=ups[:1])
                pre = []
                for w in waits[:-1]:
                    _split_ctr[0] += 1
                    n = mybir.InstNoOp(name=f"waitsplit-{_split_ctr[0]}",
                                       ins=[], outs=[])
                    n.engine = inst.engine
                    n.sync_info = bass_rust.SyncInfo(on_wait=[w], on_update=[])
                    pre.append(n)
                post = []
                for u in ups[1:]:
                    _split_ctr[0] += 1
                    n = mybir.InstNoOp(name=f"upsplit-{_split_ctr[0]}",
                                       ins=[], outs=[])
                    n.engine = inst.engine
                    n.sync_info = bass_rust.SyncInfo(on_wait=[], on_update=[u])
                    post.append(n)
                insts[i:i] = pre
                insts[i + len(pre) + 1:i + len(pre) + 1] = post
                i += len(pre) + 1 + len(post)


def _patched_drain_and_barrier(self, tick_clock, wait_clock):
    nc = self.nc
    probe = nc.sync.nop(nofuse=True, hint="drain_wait_split")
    wait_clock.add_sem_waits(
        probe.ins, ScopedClock({None: tick_clock.global_clock}))
    si = probe.ins.sync_info
    waits = list(si.on_wait) if si is not None and si.on_wait else []
    if len(waits) > 1:
        si.on_wait = [waits[0]]
        for w in waits[1:]:
            n = nc.sync.nop(nofuse=True, hint="drain_wait_split")
            n.ins.sync_info = bass_rust.SyncInfo(on_wait=[w], on_update=[])
    nc.sync.drain()
    nc.all_engine_barrier()
    assert self.sems is not None
    popped = nc._tile_sem_poison_stack.pop()
    assert popped is self._sem_poison
    nc.clear_and_free_semaphores(list(self.sems.allocated().values()))
    nc.all_engine_barrier()


tile_mod.TileContext._drain_and_barrier = _patched_drain_and_barrier


# ---------------------------------------------------------------------------
def _build_nc():
    nc = bacc.Bacc()
    dt = nc.dram_tensor
    emb_cat = dt("emb_cat", [2 * V, E], BF16, kind="ExternalInput")
    sp_tabs = [dt(f"sp{i}", [V, E], F32, kind="ExternalInput")
               for i in range(NS)]
    idx_seq = dt("idx_seq", [128, 2 * L * NBT], I32, kind="ExternalInput")
    idx_item = dt("idx_item", [128, 2 * NBT], I32, kind="ExternalInput")
    idx_sp = dt("idx_sp", [128, NS * NBT], I32, kind="ExternalInput")
    idx0lb = dt("idx0lb", [L, BC], I32, kind="ExternalInput")
    dense_t = dt("dense_t", [ND, BC], F32R, kind="ExternalInput")
    aW1 = dt("aW1", [128, 3, 80], BF16, kind="ExternalInput")
    aB1 = dt("aB1", [80, 1], F32, kind="ExternalInput")
    aW2 = dt("aW2", [80, 40], BF16, kind="ExternalInput")
    aB2 = dt("aB2", [40, 1], F32, kind="ExternalInput")
    aWf = dt("aWf", [40, 1], BF16, kind="ExternalInput")
    fW1 = dt("fW1", [456, 80], F32R, kind="ExternalInput")
    fB1 = dt("fB1", [80, 1], F32, kind="ExternalInput")
    fA1 = dt("fA1", [80, 1], F32, kind="ExternalInput")
    fW2 = dt("fW2", [80, 40], F32R, kind="ExternalInput")
    fB2 = dt("fB2", [40, 1], F32, kind="ExternalInput")
    fA2 = dt("fA2", [40, 1], F32, kind="ExternalInput")
    fWo = dt("fWo", [40, 1], F32R, kind="ExternalInput")
    fBo = dt("fBo", [1, 1], F32, kind="ExternalInput")
    selS = dt("selS", [L, 128 * L], BF16, kind="ExternalInput")
    ones40 = dt("ones40", [L, 1], BF16, kind="ExternalInput")
    onesK1 = dt("onesK1", [1, 128], F32, kind="ExternalInput")
    y = dt("y", [BC, 1], F32, kind="ExternalOutput")

    with TileContext(nc) as tc:
        with tc.tile_pool(name="sb", bufs=1) as pool:
            # --- static loads -------------------------------------------
            t_idx_seq = pool.tile([128, 2 * L * NBT], I32, tag="t_idx_seq")
            t_idx_item = pool.tile([128, 2 * NBT], I32, tag="t_idx_item")
            t_idx_sp = pool.tile([128, NS * NBT], I32, tag="t_idx_sp")
            t_idx0 = pool.tile([L, BC], I32, tag="t_idx0")
            nc.sync.dma_start(t_idx_seq[:], idx_seq[:])
            nc.sync.dma_start(t_idx_item[:], idx_item[:])
            nc.sync.dma_start(t_idx_sp[:], idx_sp[:])
            nc.sync.dma_start(t_idx0[:], idx0lb[:])

            t_aW1 = pool.tile([128, 3, 80], BF16, tag="t_aW1")
            t_aB1 = pool.tile([80, 1], F32, tag="t_aB1")
            t_aW2 = pool.tile([80, 40], BF16, tag="t_aW2")
            t_aB2 = pool.tile([40, 1], F32, tag="t_aB2")
            t_aWf = pool.tile([40, 1], BF16, tag="t_aWf")
            for t, s in [(t_aW1, aW1), (t_aB1, aB1), (t_aW2, aW2),
                         (t_aB2, aB2), (t_aWf, aWf)]:
                nc.sync.dma_start(t[:], s[:])
            t_fW1 = [pool.tile([128, 80], F32R, tag=f"t_fW1_{k}", name=f"t_fW1_{k}")
                     for k in range(3)] + [pool.tile([72, 80], F32R, tag="t_fW1_3", name="t_fW1_3")]
            for k in range(3):
                nc.sync.dma_start(t_fW1[k][:], fW1[128 * k:128 * (k + 1), :])
            nc.sync.dma_start(t_fW1[3][:], fW1[384:456, :])
            t_fB1 = pool.tile([80, 1], F32, tag="t_fB1")
            t_fA1 = pool.tile([80, 1], F32, tag="t_fA1")
            t_fW2 = pool.tile([80, 40], F32R, tag="t_fW2")
            t_fB2 = pool.tile([40, 1], F32, tag="t_fB2")
            t_fA2 = pool.tile([40, 1], F32, tag="t_fA2")
            t_fWo = pool.tile([40, 1], F32R, tag="t_fWo")
            t_fBo = pool.tile([1, 1], F32, tag="t_fBo")
            for t, s in [(t_fB1, fB1), (t_fA1, fA1), (t_fW2, fW2),
                         (t_fB2, fB2), (t_fA2, fA2), (t_fWo, fWo),
                         (t_fBo, fBo)]:
                nc.sync.dma_start(t[:], s[:])

            idb = pool.tile([128, 128], BF16, tag="idb")
            idf = pool.tile([128, 128], F32, tag="idf")
            make_identity(nc, idb[:])
            make_identity(nc, idf[:])

            # xT3 holds sparse2^T plus dense^T rows; dense lands directly.
            xT = [pool.tile([128, BC], F32R, tag=f"xT{k}", name=f"xT{k}") for k in range(3)]
            xT.append(pool.tile([72, BC], F32R, tag="xT3", name="xT3"))
            nc.sync.dma_start(xT[3][64:72, :], dense_t[:])

            # persistent gathered data
            seq = pool.tile([128, L * NBT, 2 * E], BF16, tag="seq")
            item = pool.tile([128, 2 * NBT, E], BF16, tag="item")
            spr = pool.tile([128, NS * NBT, E], F32, tag="spr")

            maskF = pool.tile([L, BC], BF16, tag="maskF")
            nc.vector.tensor_scalar(out=maskF[:], in0=t_idx0[:], scalar1=0,
                                    scalar2=None, op0=AluOpType.not_equal)

            t_selS = pool.tile([L, 128 * L], BF16, tag="t_selS")
            t_ones40 = pool.tile([L, 1], BF16, tag="t_ones40")
            t_onesK1 = pool.tile([1, 128], F32, tag="t_onesK1")
            nc.sync.dma_start(t_selS[:], selS[:])
            nc.sync.dma_start(t_ones40[:], ones40[:])
            nc.sync.dma_start(t_onesK1[:], onesK1[:])
            emLB = pool.tile([L, BC], BF16, tag="emLB")
            nc.vector.memset(emLB[:], 0.0)
            uiT_un = pool.tile([128, BC], F32, tag="uiT_un")

            # --- gathers: item + sparse (small) --------------------------
            for c in range(2 * NBT):
                nc.gpsimd.indirect_dma_start(
                    out=item[:, c, :], out_offset=None, in_=emb_cat[:],
                    in_offset=bass.IndirectOffsetOnAxis(
                        ap=t_idx_item[:, c:c + 1], axis=0))

            with tc.tile_pool(name="psA", bufs=1, space="PSUM") as psA:
                # qT: [128 feat, BC] bf16 from item tiles
                qT = pool.tile([128, BC], BF16, tag="qT")
                for tb in range(2):
                    for bt in range(NBT):
                        pt = psA.tile([128, 128], BF16, tag="ptrans", bufs=2)
                        nc.tensor.transpose(
                            pt[0:64, :], item[:, tb * NBT + bt, :], idb[:])
                        nc.vector.tensor_copy(
                            qT[64 * tb:64 * (tb + 1),
                               bt * 128:(bt + 1) * 128], pt[0:64, :])

                # --- main attention loop over l --------------------------
                for l in range(L):
                    for tb in range(2):
                        for bt in range(NBT):
                            j = l * NBT + bt
                            nc.gpsimd.indirect_dma_start(
                                out=seq[:, j, tb * E:(tb + 1) * E],
                                out_offset=None, in_=emb_cat[:],
                                in_offset=bass.IndirectOffsetOnAxis(
                                    ap=t_idx_seq[:, tb * L * NBT + j:
                                                 tb * L * NBT + j + 1],
                                    axis=0))
                    sT = pool.tile([128, BC], BF16, tag="sT", bufs=3)
                    for bt in range(NBT):
                        pt = psA.tile([128, 128], BF16, tag="ptrans", bufs=2)
                        nc.tensor.transpose(pt[:], seq[:, l * NBT + bt, :],
                                            idb[:])
                        nc.vector.tensor_copy(
                            sT[:, bt * 128:(bt + 1) * 128], pt[:])
                    qs = pool.tile([128, BC], BF16, tag="qs", bufs=2)
                    h1 = pool.tile([80, BC], BF16, tag="h1", bufs=2)
                    h2 = pool.tile([40, BC], BF16, tag="h2", bufs=2)
                    pfl = psA.tile([33, 512], F32, tag="pfl", bufs=1)
                    ftmp = pool.tile([33, 512], BF16, tag="ftmp", bufs=2)
                    for ns in range(2):
                        sl = slice(512 * ns, 512 * (ns + 1))
                        nc.vector.tensor_tensor(
                            out=qs[:, sl], in0=sT[:, sl], in1=qT[:, sl],
                            op=AluOpType.mult)
                        p1 = psA.tile([80, 512], F32, tag="pmm1", bufs=2)
                        nc.tensor.matmul(p1[:], t_aW1[:, 0, :], qT[:, sl],
                                         start=True, stop=False)
                        nc.tensor.matmul(p1[:], t_aW1[:, 1, :], sT[:, sl],
                                         start=False, stop=False)
                        nc.tensor.matmul(p1[:], t_aW1[:, 2, :], qs[:, sl],
                                         start=False, stop=True)
                        nc.scalar.activation(
                            h1[:, sl], p1[:],
                            mybir.ActivationFunctionType.Sigmoid,
                            bias=t_aB1[:])
                        p2 = psA.tile([40, 512], F32, tag="pmm2", bufs=1)
                        nc.tensor.matmul(p2[:], t_aW2[:], h1[:, sl],
                                         start=True, stop=True)
                        nc.scalar.activation(
                            h2[:, sl], p2[:],
                            mybir.ActivationFunctionType.Sigmoid,
                            bias=t_aB2[:])
                        nc.tensor.matmul(pfl[32 * ns:32 * ns + 1, :],
                                         t_aWf[:], h2[:, sl],
                                         start=True, stop=True,
                                         tile_position=(0, 32 * ns))
                    m0 = pool.tile([33, 512], BF16, tag="m0", bufs=2,
                                   name="m0")
                    nc.sync.dma_start(m0[0:1, :], maskF[l:l + 1, 0:512])
                    nc.sync.dma_start(m0[32:33, :],
                                      maskF[l:l + 1, 512:1024])
                    nc.scalar.activation(ftmp[:], pfl[:],
                                         mybir.ActivationFunctionType.Exp)
                    nc.vector.tensor_tensor(out=ftmp[0:1, :],
                                            in0=ftmp[0:1, :],
                                            in1=m0[0:1, :],
                                            op=AluOpType.mult)
                    nc.vector.tensor_tensor(out=ftmp[32:33, :],
                                            in0=ftmp[32:33, :],
                                            in1=m0[32:33, :],
                                            op=AluOpType.mult)
                    nc.sync.dma_start(emLB[l:l + 1, 0:512], ftmp[0:1, :])
                    nc.sync.dma_start(emLB[l:l + 1, 512:1024],
                                      ftmp[32:33, :])
                    for ns in range(2):
                        sl = slice(512 * ns, 512 * (ns + 1))
                        pe = psA.tile([128, 512], F32, tag="pemB", bufs=2,
                                      name="pemB")
                        nc.tensor.matmul(pe[:],
                                         t_selS[:, 128 * l:128 * (l + 1)],
                                         emLB[:, sl], start=True, stop=True)
                        emBsb = pool.tile([128, 512], BF16, tag="emBsb",
                                          bufs=2, name="emBsb")
                        nc.vector.tensor_copy(emBsb[:], pe[:])
                        uit = pool.tile([128, 512], F32, tag="uitmp",
                                        bufs=2, name="uitmp")
                        nc.vector.tensor_tensor(out=uit[:], in0=sT[:, sl],
                                                in1=emBsb[:],
                                                op=AluOpType.mult)
                        if l == 0:
                            nc.vector.tensor_copy(uiT_un[:, sl], uit[:])
                        else:
                            nc.vector.tensor_tensor(
                                out=uiT_un[:, sl], in0=uiT_un[:, sl],
                                in1=uit[:], op=AluOpType.add)

            for si in range(NS):
                for bt in range(NBT):
                    c = si * NBT + bt
                    nc.gpsimd.indirect_dma_start(
                        out=spr[:, c, :], out_offset=None, in_=sp_tabs[si][:],
                        in_offset=bass.IndirectOffsetOnAxis(
                            ap=t_idx_sp[:, c:c + 1], axis=0))
            # --- softmax denominator + normalize (feature-major) ---------
            with tc.tile_pool(name="psB", bufs=1, space="PSUM") as psB:
                rden0 = pool.tile([1, BC], F32, tag="rden0")
                for ns in range(2):
                    sl = slice(512 * ns, 512 * (ns + 1))
                    pden = psB.tile([1, 512], F32, tag="p1x512", bufs=2,
                                    name="pden")
                    nc.tensor.matmul(pden[:], t_ones40[:], emLB[:, sl],
                                     start=True, stop=True)
                    nc.vector.reciprocal(rden0[0:1, sl], pden[:])
                for ns in range(2):
                    sl = slice(512 * ns, 512 * (ns + 1))
                    prb = psB.tile([128, 512], F32, tag="prdenB", bufs=1,
                                   name="prb")
                    nc.tensor.matmul(prb[:], t_onesK1[:], rden0[0:1, sl],
                                     start=True, stop=True)
                    nc.vector.tensor_tensor(out=xT[0][:, sl],
                                            in0=uiT_un[:, sl], in1=prb[:],
                                            op=AluOpType.mult)
                nc.vector.tensor_copy(xT[1][:], qT[:])
                for si in range(NS):
                    for bt in range(NBT):
                        pu = psB.tile([128, 128], F32, tag="puiT", bufs=1)
                        nc.tensor.transpose(
                            pu[0:64, :], spr[:, si * NBT + bt, :], idf[:])
                        dst = (xT[2][64 * si:64 * (si + 1),
                                     bt * 128:(bt + 1) * 128] if si < 2
                               else xT[3][0:64, bt * 128:(bt + 1) * 128])
                        nc.vector.tensor_copy(dst, pu[0:64, :])

                # --- FFN ------------------------------------------------
                y_sb = pool.tile([1, BC], F32, tag="y_sb")
                fh1 = pool.tile([80, BC], F32R, tag="fh1")
                fh2 = pool.tile([40, BC], F32R, tag="fh2")
                tmp_b = pool.tile([80, 512], F32, tag="tmp_b", bufs=2)
                tmp_n = pool.tile([80, 512], F32, tag="tmp_n", bufs=2)
                tmp_p = pool.tile([80, 512], F32, tag="tmp_p", bufs=2)
                for ns in range(2):
                    sl = slice(512 * ns, 512 * (ns + 1))
                    pf1 = psB.tile([80, 512], F32, tag="pffn1", bufs=2)
                    nc.tensor.matmul(pf1[:], t_fW1[0][:], xT[0][:, sl],
                                     start=True, stop=False)
                    nc.tensor.matmul(pf1[:], t_fW1[1][:], xT[1][:, sl],
                                     start=False, stop=False)
                    nc.tensor.matmul(pf1[:], t_fW1[2][:], xT[2][:, sl],
                                     start=False, stop=False)
                    nc.tensor.matmul(pf1[:], t_fW1[3][:], xT[3][:, sl],
                                     start=False, stop=True)
                    bq = pool.tile([80, 512], F32, tag="bq", bufs=2)
                    nc.vector.tensor_scalar(
                        out=bq[:], in0=pf1[:], scalar1=t_fB1[:],
                        scalar2=None, op0=AluOpType.add)
                    nc.vector.tensor_scalar(out=tmp_n[:80], in0=bq[:],
                                            scalar1=0.0, scalar2=None,
                                            op0=AluOpType.min)
                    nc.vector.tensor_scalar(out=tmp_p[:80], in0=bq[:],
                                            scalar1=0.0, scalar2=None,
                                            op0=AluOpType.max)
                    nc.vector.scalar_tensor_tensor(
                        out=fh1[:, sl], in0=tmp_n[:80], scalar=t_fA1[:],
                        in1=tmp_p[:80], op0=AluOpType.mult,
                        op1=AluOpType.add)
                    pf2 = psB.tile([40, 512], F32, tag="pffn2", bufs=1)
                    nc.tensor.matmul(pf2[:], t_fW2[:], fh1[:, sl],
                                     start=True, stop=True)
                    bq2 = pool.tile([40, 512], F32, tag="bq2", bufs=2)
                    nc.vector.tensor_scalar(
                        out=bq2[:], in0=pf2[:], scalar1=t_fB2[:],
                        scalar2=None, op0=AluOpType.add)
                    nc.vector.tensor_scalar(out=tmp_n[:40], in0=bq2[:],
                                            scalar1=0.0, scalar2=None,
                                            op0=AluOpType.min)
                    nc.vector.tensor_scalar(out=tmp_p[:40], in0=bq2[:],
                                            scalar1=0.0, scalar2=None,
                                            op0=AluOpType.max)
                    nc.vector.scalar_tensor_tensor(
                        out=fh2[:, sl], in0=tmp_n[:40], scalar=t_fA2[:],
                        in1=tmp_p[:40], op0=AluOpType.mult,
                        op1=AluOpType.add)
                    pf3 = psB.tile([1, 512], F32, tag="pffn3", bufs=1)
                    nc.tensor.matmul(pf3[:], t_fWo[:], fh2[:, sl],
                                     start=True, stop=True)
                    nc.scalar.activation(
                        y_sb[0:1, sl], pf3[:],
                        mybir.ActivationFunctionType.Sigmoid,
                        bias=t_fBo[:])
            nc.sync.dma_start(y[:].rearrange("a b -> b a"), y_sb[:])

    nc.compile()
    _split_multi_waits(nc)
    bass.Bass.finalize(nc)
    return nc


_NC_CACHE = []


def _get_nc():
    if not _NC_CACHE:
        _NC_CACHE.append(_build_nc())
    return _NC_CACHE[0]


_SELS = np.zeros((L, 128 * L), np.float32)
for _l in range(L):
    _SELS[_l, 128 * _l:128 * (_l + 1)] = 1.0


def _prep_shared(inputs):
    emb_cat = np.ascontiguousarray(
        np.concatenate([np.asarray(inputs["emb_seq"][0]),
                        np.asarray(inputs["emb_seq"][1])], axis=0)
    ).astype(BF16NP)
    sp = [np.ascontiguousarray(np.asarray(inputs["emb_sparse"][i]),
                               dtype=np.float32) for i in range(NS)]

    W1 = np.asarray(inputs["att_W1"], np.float32)          # [512, 80]
    W1q = W1[0:128] + W1[256:384]
    W1s = W1[128:256] - W1[256:384]
    W1qs = W1[384:512]
    aW1 = np.stack([W1q, W1s, W1qs], axis=1).astype(BF16NP)  # [128, 3, 80]
    aB1 = np.asarray(inputs["att_b1"], np.float32).reshape(80, 1)
    aW2 = np.asarray(inputs["att_W2"], np.float32).astype(BF16NP)
    aB2 = np.asarray(inputs["att_b2"], np.float32).reshape(40, 1)
    aWf = np.asarray(inputs["att_Wf"], np.float32).astype(BF16NP)

    gamma = np.asarray(inputs["bn_gamma"], np.float32)
    beta = np.asarray(inputs["bn_beta"], np.float32)
    mean = np.asarray(inputs["bn_mean"], np.float32)
    var = np.asarray(inputs["bn_var"], np.float32)
    scale = gamma / np.sqrt(var + 1e-3)
    shift = beta - mean * scale
    fW1o = np.asarray(inputs["ffn_W1"], np.float32)        # [456, 80]
    W1f = fW1o * scale[:, None]
    b1f = shift @ fW1o + np.asarray(inputs["ffn_b1"], np.float32)
    # reorder rows [ui(0:128), item(128:256), dense(256:264), sparse(264:456)]
    # -> [ui, item, sparse, dense]
    perm = np.concatenate([np.arange(0, 256), np.arange(264, 456),
                           np.arange(256, 264)])
    fW1 = np.ascontiguousarray(W1f[perm])
    return dict(
        emb_cat=emb_cat, sp=sp, aW1=aW1, aB1=aB1, aW2=aW2, aB2=aB2, aWf=aWf,
        fW1=fW1, fB1=b1f.reshape(80, 1),
        fA1=np.asarray(inputs["ffn_a1"], np.float32).reshape(80, 1),
        fW2=np.asarray(inputs["ffn_W2"], np.float32),
        fB2=np.asarray(inputs["ffn_b2"], np.float32).reshape(40, 1),
        fA2=np.asarray(inputs["ffn_a2"], np.float32).reshape(40, 1),
        fWo=np.asarray(inputs["out_W"], np.float32),
        fBo=np.asarray(inputs["out_b"], np.float32).reshape(1, 1),
        selS=_SELS.astype(BF16NP), ones40=np.ones((L, 1), np.float32).astype(BF16NP),
        onesK1=np.ones((1, 128), np.float32),
    )


def _prep_core(inputs, ci):
    s = slice(ci * BC, (ci + 1) * BC)
    seq = np.asarray(inputs["seq_inputs"])[s]      # [1024, 40, 2]
    itm = np.asarray(inputs["item_inputs"])[s]     # [1024, 2]
    spi = np.asarray(inputs["sparse_inputs"])[s]   # [1024, 3]
    dns = np.asarray(inputs["dense_inputs"])[s]    # [1024, 8]

    idx_seq = np.empty((128, 2 * L * NBT), np.int32)
    for tb in range(2):
        # [1024, 40] -> cols (l*8 + bt), partition = b % 128
        v = seq[:, :, tb].reshape(NBT, 128, L) + tb * V
        idx_seq[:, tb * L * NBT:(tb + 1) * L * NBT] = (
            v.transpose(1, 2, 0).reshape(128, L * NBT))
    idx_item = np.empty((128, 2 * NBT), np.int32)
    for tb in range(2):
        idx_item[:, tb * NBT:(tb + 1) * NBT] = (
            itm[:, tb].reshape(NBT, 128).T + tb * V)
    idx_sp = np.empty((128, NS * NBT), np.int32)
    for si in range(NS):
        idx_sp[:, si * NBT:(si + 1) * NBT] = spi[:, si].reshape(NBT, 128).T
    idx0lb = np.ascontiguousarray(seq[:, :, 0].T.astype(np.int32))  # [40,1024]
    dense_t = np.ascontiguousarray(dns.T)                  # [8, 1024]
    return dict(idx_seq=idx_seq, idx_item=idx_item, idx_sp=idx_sp,
                idx0lb=idx0lb, dense_t=dense_t)


def kernel(**inputs):
    nc = _get_nc()
    sh = _prep_shared(inputs)
    in_maps = []
    for ci in range(NCORES):
        pc = _prep_core(inputs, ci)
        m = {
            "emb_cat": np.asarray(sh["emb_cat"]),
            "sp0": sh["sp"][0], "sp1": sh["sp"][1], "sp2": sh["sp"][2],
            "idx_seq": pc["idx_seq"], "idx_item": pc["idx_item"],
            "idx_sp": pc["idx_sp"], "idx0lb": pc["idx0lb"],
            "dense_t": pc["dense_t"],
            "aW1": np.asarray(sh["aW1"]), "aB1": sh["aB1"],
            "aW2": np.asarray(sh["aW2"]), "aB2": sh["aB2"],
            "aWf": np.asarray(sh["aWf"]),
            "fW1": sh["fW1"], "fB1": sh["fB1"], "fA1": sh["fA1"],
            "fW2": sh["fW2"], "fB2": sh["fB2"], "fA2": sh["fA2"],
            "fWo": sh["fWo"], "fBo": sh["fBo"],
            "selS": np.asarray(sh["selS"]), "ones40": np.asarray(sh["ones40"]),
            "onesK1": sh["onesK1"],
        }
        in_maps.append(m)
    res = run_bass_kernel_spmd(nc, in_maps, core_ids=list(range(NCORES)))
    out = np.concatenate([res.results[ci]["y"] for ci in range(NCORES)],
                         axis=0)
    return out
